# revision 22
# baseline (speedup 1.0000x reference)
"""Trainium2 Bass kernel for nn_NetNew_17162689315115 (dense_mlp), v2.

Network: 8 layers of  h <- concat(ops(W_i @ h), h)  starting h = x [B, 8],
then y = h @ Wf.T.  ops = 9 columns: +, -, *(clip 1e8), /(clip 9999),
sin, cos, exp(cap 17), log|.|, square(clip 1e8), consuming 13 z-columns.

v2 design (vs v1 "quartered contribution-form" baseline):
- Data parallel over 8 cores (65536 rows each); per core 16 supergroups
  of 32 chunk-slots x 128 rows.
- PSUM mega-tile [128, 4096]: slot t's z-future strip at cols 128t..+104
  (104 = 8*13 z cols; Wf handled off-PE).  No zero-fill: first block's
  matmuls use start=True.
- Matmuls stay quartered fp32 contribution-form, but each layer-step's
  stream for the last NSPLIT slots is split part1 (the 13 z cols the next
  ops need) / part2 (future cols) so the PE keeps streaming while the ops
  engines work: no per-layer PE stall.
- Ops engines (per layer, all 32 slots at once, free=32):
  * ACT (one table set: natural_log_exp_and_others): binary-col copy,
    Exp, Ln, Square  (exp/ln tables replace v1's 33-instruction software
    exp/log chains).
  * DVE: reciprocal (div), sin/cos via 4 fused custom-DVE ops each
    (magic-round, Cody-Waite cascade, two poly stages; deg-11/12 minimax,
    ~3e-7 max err), exp clamp, ln |x| bit-mask, and the 32x32 block
    transposes.
  * Pool (GpSimd): +, -, *, clips, and the Wf dot-product accumulation
    (h@Wf.T computed incrementally from batch-major ops outputs, so
    layer-8 ops are never transposed and the PE never streams Wf cols).
- Unary ops read z directly from PSUM (single-PSUM-operand rule); binary
  ops read an ACT-copied SBUF staging tile.  Layer 8 reads an SBUF slab
  so the PSUM tile is released early for the next supergroup.
"""
import numpy as np
import concourse.bass as bass
import concourse.tile as tile
from concourse import bacc, mybir
from concourse.bass_utils import run_bass_kernel_spmd
from concourse.dve_spec import Spec, Src0, Src1, C0, C1, C2, sq, lower, _has_src1
from concourse.dve_ops import DveOp, OPS, _SUB_OPCODE_FOR_NAME, CUSTOM_DVE_SPECS
from concourse.dve_uop import DveOpSpec

f32 = mybir.dt.float32
i32 = mybir.dt.int32
AF = mybir.ActivationFunctionType
ALU = mybir.AluOpType

B_FULL = 524288
N_CORES = 8
BC = B_FULL // N_CORES          # rows per core
T = 32                          # chunk-slots (128 rows) per supergroup
ROWS_PER_G = 128 * T            # 4096
NG = BC // ROWS_PER_G           # 16 supergroups per core
NSPLIT = 8                      # slots per step emitted split part1/part2

# ---- matmul blocks: j=0 is x (K=8), j=1..7 are ops_j (K=9); ops_8 never
# touches the PE. Block j streams z-future cols 13j..104 plus the Wf col
# (104), so h@Wf.T accumulates in PSUM too (except the ops_8 part, done on
# DVE at pair end).
BLK_K = [8] + [9] * 7
BLK_N = [104 - 13 * j + 1 for j in range(8)]        # 105, 92, ..., 14
BLK_OFF = np.concatenate([[0], np.cumsum(BLK_N)]).astype(int)
WS_COLS = int(BLK_OFF[-1])                          # 476

# ---- numeric constants ----
TWO_PI = 2.0 * np.pi
INV_2PI = float(np.float32(1.0 / TWO_PI))
MAGIC = 12582912.0            # 1.5 * 2^23 round-to-nearest trick


def _trunc_f32(v, keep_bits):
    u = np.frombuffer(np.float32(v).tobytes(), dtype=np.uint32)[0]
    mask = np.uint32(0xFF800000) | np.uint32(((1 << keep_bits) - 1) << (23 - keep_bits))
    u = np.uint32(u & mask)
    return float(np.frombuffer(u.tobytes(), dtype=np.float32)[0])


CW1 = _trunc_f32(TWO_PI, 8)
CW2 = _trunc_f32(TWO_PI - CW1, 8)
CW3 = float(np.float32(TWO_PI - CW1 - CW2))


def _fit_trig():
    th = np.linspace(0, np.pi, 300001)
    def fit(target, powers):
        A = th[:, None] ** powers[None, :]
        c, *_ = np.linalg.lstsq(A, target, rcond=None)
        return [float(np.float32(v)) for v in c]
    sinc = fit(np.sin(th), np.arange(1, 12, 2))     # s1 s3 s5 s7 s9 s11
    cosc = fit(np.cos(th), np.arange(0, 13, 2))     # c0 c2 c4 c6 c8 c10 c12
    return sinc, cosc


SINC, COSC = _fit_trig()
LOG2E = float(np.float32(np.log2(np.e)))


def _fit_exp2():
    fgrid = np.linspace(-0.5, 0.5, 20001)
    ch = np.polynomial.chebyshev.Chebyshev.fit(fgrid, np.exp2(fgrid), 5)
    p = ch.convert(kind=np.polynomial.Polynomial)
    return [float(np.float32(c)) for c in p.coef]          # c0..c5


EXP_C = _fit_exp2()

# ---- custom DVE ops (registered into dve_ops at import) ----


def _register_op(name, spec):
    for o in OPS:
        if o.name == name:
            return o
    row = max(_SUB_OPCODE_FOR_NAME.values()) + 1
    _SUB_OPCODE_FOR_NAME[name] = row
    shas = {}
    for ver in ("v3", "v4"):
        try:
            ds = DveOpSpec(name=name, opcode=row, uops=lower(spec, ver=ver),
                           rd1_en=_has_src1(spec))
            shas[ver] = ds.sha(ver)
        except Exception:
            pass
    op = DveOp(name, spec, subdim=False, uops_sha=shas)
    OPS.append(op)
    CUSTOM_DVE_SPECS[name] = spec
    return op


def _np32(x):
    return np.float32(x)


# k = (x*C0 + C1) - C1  (round-to-nearest via 1.5*2^23 magic)
ANT_RED_K = _register_op("ANT_RED_K", Spec(
    body=(Src0 * C0 + C1) - C1,
    reference=lambda in0, s0, s1, imm2: (
        _np32(_np32(in0 * _np32(s0)) + _np32(s1)) - _np32(s1)),
))

# out = ((u*Src1 + C0)*u + C1)*u + C2  with u = Src0^2  (poly high part)
_u0 = sq(Src0)
ANT_POLY_A = _register_op("ANT_POLY_A", Spec(
    body=((_u0 * Src1 + C0) * _u0 + C1) * _u0 + C2,
    reference=lambda in0, in1, s0, s1, imm2: (
        ((in0 * in0 * in1 + s0) * (in0 * in0) + s1) * (in0 * in0) + imm2),
))

# out = ((Src0*u + C0)*u + C1) * Src1  with u = Src1^2  (odd poly finish)
_u1 = sq(Src1)
ANT_POLY_B_ODD = _register_op("ANT_POLY_B_ODD", Spec(
    body=((Src0 * _u1 + C0) * _u1 + C1) * Src1,
    reference=lambda in0, in1, s0, s1, imm2: (
        ((in0 * (in1 * in1) + s0) * (in1 * in1) + s1) * in1),
))

# out = ((Src0*u + C0)*u + C1)*u + C2  with u = Src1^2  (even poly finish)
ANT_POLY_B_EVEN = _register_op("ANT_POLY_B_EVEN", Spec(
    body=((Src0 * _u1 + C0) * _u1 + C1) * _u1 + C2,
    reference=lambda in0, in1, s0, s1, imm2: (
        ((in0 * (in1 * in1) + s0) * (in1 * in1) + s1) * (in1 * in1) + imm2),
))

# out = (Src0*C0 + C1)*Src0 + C2   (plain Horner head, deg 2)
ANT_H3A = _register_op("ANT_H3A", Spec(
    body=(Src0 * C0 + C1) * Src0 + C2,
    reference=lambda in0, s0, s1, imm2: (in0 * s0 + s1) * in0 + imm2,
))

# out = ((Src0*Src1 + C0)*Src1 + C1)*Src1 + C2   (Horner tail, 3 more levels)
ANT_HT3 = _register_op("ANT_HT3", Spec(
    body=((Src0 * Src1 + C0) * Src1 + C1) * Src1 + C2,
    reference=lambda in0, in1, s0, s1, imm2: (
        ((in0 * in1 + s0) * in1 + s1) * in1 + imm2),
))

_PROG_CACHE = {}
DEBUG_TAP = False


def _build_wstream(Ws, Wf):
    """[128, 476] quarter-replicated contribution weight streams (fp32),
    blocks j=0..7, z-future cols + the block's Wf column."""
    ws = np.zeros((128, WS_COLS), np.float32)
    for j in range(8):
        K = BLK_K[j]
        parts = []
        for t in range(j + 1, 9):
            Wt = Ws[t - 1]                     # W_t: [13, 8 + 9*(t-1)]
            if j == 0:
                sl = Wt[:, 9 * (t - 1): 9 * (t - 1) + 8]
            else:
                sl = Wt[:, 9 * (t - 1 - j): 9 * (t - 1 - j) + 9]
            parts.append(sl.T.astype(np.float32))            # [K, 13]
        if j == 0:
            parts.append(Wf[:, 72:80].T.astype(np.float32))  # [8, 1]
        else:
            parts.append(Wf[:, 9 * (8 - j): 9 * (9 - j)].T.astype(np.float32))
        blk = np.concatenate(parts, axis=1)                  # [K, Nj]
        assert blk.shape == (K, BLK_N[j]), (blk.shape, K, BLK_N[j])
        off = BLK_OFF[j]
        for a in range(4):
            ws[32 * a: 32 * a + K, off: off + BLK_N[j]] = blk
    return ws


def _build_wf(Wf):
    """[128, 80] Wf row broadcast down partitions.
    Wf col order: ops8(0..8), ops7(9..17), ..., ops1(63..71), x(72..79)."""
    return np.broadcast_to(Wf[0:1, :], (128, 80)).astype(np.float32).copy()


def _emit_ops(nc, spool, zf4, bc3, slab3, ot3, acc, wf, consts, layer, half):
    """ops for one layer over one 16-slot half (free=16).

    zf4:  PSUM [128, 16, 128] half-slice, bc3: SBUF [128, 16, 8] staging of
    the binary cols, slab3: SBUF [128, 16, 13] (layer 8 only), ot3:
    [128, 16, 32] output, acc: [128, 16] Wf accumulator half."""
    bs11, bc12 = consts
    i = layer

    def S(c):
        if i == 8:
            return slab3[:, :, c]
        return zf4[:, :, 13 * (i - 1) + c]

    def BCc(c):
        if i == 8:
            return slab3[:, :, c]
        return bc3[:, :, c]

    def D(c):
        return ot3[:, :, c]

    _seq = [0]

    def TT():
        t_scr = spool.tile([128, 16], f32, tag="scr",
                           name=f"scr{half}_{_seq[0]}")
        _seq[0] += 1
        return t_scr

    v = nc.vector
    g = nc.gpsimd
    s = nc.scalar

    # sin (deg-11 odd) / cos (deg-12 even) first: direct PSUM reads, no ACT
    # staging hop on the critical path.
    for (src, dst, isin) in ((S(8), D(4), True), (S(9), D(5), False)):
        k = TT()
        v._custom_dve(ANT_RED_K, out=k, in0=src, s0=INV_2PI, s1=MAGIC)
        th = TT()
        v.cody_waite_cascade(th, src, k, CW1, CW2, CW3)
        pa = TT()
        if isin:
            v._custom_dve(ANT_POLY_A, out=pa, in0=th, in1=bs11,
                          s0=SINC[4], s1=SINC[3], imm2=SINC[2])
            v._custom_dve(ANT_POLY_B_ODD, out=dst, in0=pa, in1=th,
                          s0=SINC[1], s1=SINC[0])
        else:
            v._custom_dve(ANT_POLY_A, out=pa, in0=th, in1=bc12,
                          s0=COSC[5], s1=COSC[4], imm2=COSC[3])
            v._custom_dve(ANT_POLY_B_EVEN, out=dst, in0=pa, in1=th,
                          s0=COSC[2], s1=COSC[1], imm2=COSC[0])
    # exp on DVE: 2^(x*log2e) with magic-round split + deg-5 poly.
    e1 = TT()
    v.tensor_scalar(e1, S(10), 17.0, -87.0, ALU.min, ALU.max)
    en = TT()
    v._custom_dve(ANT_RED_K, out=en, in0=e1, s0=LOG2E, s1=MAGIC)
    ef = TT()
    v.scalar_tensor_tensor(ef, e1, LOG2E, en, ALU.mult, ALU.subtract)
    ehi = TT()
    v._custom_dve(ANT_H3A, out=ehi, in0=ef, s0=EXP_C[5], s1=EXP_C[4],
                  imm2=EXP_C[3])
    ep = TT()
    v._custom_dve(ANT_HT3, out=ep, in0=ehi, in1=ef, s0=EXP_C[2],
                  s1=EXP_C[1], imm2=EXP_C[0])
    eni = TT()
    v.tensor_copy(eni.bitcast(i32), en)
    enb = TT()
    v.tensor_scalar(enb.bitcast(i32), eni.bitcast(i32), 127, None, ALU.add)
    ebits = TT()
    v.tensor_scalar(ebits.bitcast(i32), enb.bitcast(i32), 23, None,
                    ALU.arith_shift_left)
    g.tensor_tensor(D(6), ep, ebits, ALU.mult)
    # ln|x|: DVE abs-bits (direct PSUM) + ACT Ln
    la = TT()
    v.tensor_scalar(la.bitcast(i32), S(11).bitcast(i32), 0x7FFFFFFF, None,
                    ALU.bitwise_and)
    s.activation(D(7), la, AF.Ln)
    # square: ACT Square (direct PSUM) + DVE clip
    sqv = TT()
    s.activation(sqv, S(12), AF.Square)
    v.tensor_scalar(D(8), sqv, 99999999.0, None, ALU.min)
    # binary ops from the ACT-staged bc tile: Pool tensor_tensor + DVE clips
    g.tensor_tensor(D(0), BCc(0), BCc(1), ALU.add)
    g.tensor_tensor(D(1), BCc(2), BCc(3), ALU.subtract)
    m = TT()
    g.tensor_tensor(m, BCc(4), BCc(5), ALU.mult)
    v.tensor_scalar(D(2), m, -99999999.0, 99999999.0, ALU.max, ALU.min)
    r1, r2, q = TT(), TT(), TT()
    v.reciprocal_approx_accurate(r1, BCc(7), r2)
    g.tensor_tensor(q, BCc(6), r1, ALU.mult)
    v.tensor_scalar(D(3), q, -9999.0, 9999.0, ALU.max, ALU.min)
    # Wf accumulation on DVE for layer 8 only (other layers' Wf parts ride
    # the PE contribution streams into zf col 104)
    if i == 8:
        for c in range(9):
            src_acc = acc if c else zf4[:, :, 104]
            v.scalar_tensor_tensor(acc, D(c), wf[:, c: c + 1], src_acc,
                                   ALU.mult, ALU.add)


def _build_program(bc=BC, t_slots=T, ng=NG, debug=False):
    nc = bacc.Bacc("TRN2", target_bir_lowering=False)
    x_d = nc.dram_tensor("x", [bc, 8], f32, kind="ExternalInput")
    w_d = nc.dram_tensor("ws", [128, WS_COLS], f32, kind="ExternalInput")
    wf_d = nc.dram_tensor("wf", [128, 80], f32, kind="ExternalInput")
    y_d = nc.dram_tensor("y", [bc, 1], f32, kind="ExternalOutput")
    dbg = {}
    if debug:
        for i in range(1, 9):
            dbg[f"ot{i}"] = nc.dram_tensor(f"o_ot{i}", [128, 32 * t_slots], f32,
                                           kind="ExternalOutput")
        dbg["zf"] = nc.dram_tensor("o_zf", [128, 128 * t_slots], f32,
                                   kind="ExternalOutput")

    x_r = x_d.ap().rearrange("(g t p) f -> p g t f", p=128, t=t_slots)
    y_r = y_d.ap().rearrange("(g t p) o -> p g t o", p=128, t=t_slots)

    with tile.TileContext(nc) as tc:
        with tc.tile_pool(name="const", bufs=1) as cpool, \
             tc.tile_pool(name="x", bufs=2) as xpool, \
             tc.tile_pool(name="q", bufs=3) as qpool, \
             tc.tile_pool(name="o", bufs=2) as opool, \
             tc.tile_pool(name="bc", bufs=2) as bcpool, \
             tc.tile_pool(name="slab", bufs=2) as slpool, \
             tc.tile_pool(name="scr", bufs=56) as spool, \
             tc.tile_pool(name="fin", bufs=2) as fpool, \
             tc.tile_pool(name="z", bufs=1, space="PSUM") as zpool:

            wtile = cpool.tile([128, WS_COLS], f32)
            nc.sync.dma_start(wtile[:], w_d.ap())
            wf = cpool.tile([128, 80], f32)
            nc.sync.dma_start(wf[:], wf_d.ap())
            # full-shape coeff tiles: a [P,1]-broadcast Src1 faults the DVE
            # (probe-verified); full-shape Src1 is bit-exact.
            bs11 = cpool.tile([128, 16], f32)
            nc.vector.memset(bs11[:], SINC[5])
            bc12 = cpool.tile([128, 16], f32)
            nc.vector.memset(bc12[:], COSC[6])
            consts = (bs11[:], bc12[:])
            bf16 = mybir.dt.bfloat16
            zl = cpool.tile([1, 128], bf16)
            nc.vector.memset(zl[:], 0.0)
            zr = cpool.tile([1, 512], bf16)
            nc.vector.memset(zr[:], 0.0)

            H = t_slots // 2
            NSPL = 6                      # split slots per half

            def emit_fp1(j, q, zf, t0, t1, qbase):
                """block j's full streams + part1s for slots [t0, t1)."""
                K, off, Nj = BLK_K[j], int(BLK_OFF[j]), BLK_N[j]
                last = (j == 7)
                plain_end = t1 - NSPL if Nj > 13 else t1
                for t in range(t0, plain_end):
                    base = 128 * t + 13 * j
                    qc = 32 * (t - qbase)
                    for a in range(4):
                        nc.tensor.matmul(
                            zf[32 * a: 32 * a + 32, base: base + Nj],
                            lhsT=q[32 * a: 32 * a + K, qc: qc + 32],
                            rhs=wtile[32 * a: 32 * a + K, off: off + Nj],
                            start=False, stop=last,
                            tile_position=(32 * a, 32 * a))
                for t in range(plain_end, t1):
                    base = 128 * t + 13 * j
                    qc = 32 * (t - qbase)
                    for a in range(4):
                        nc.tensor.matmul(
                            zf[32 * a: 32 * a + 32, base: base + 13],
                            lhsT=q[32 * a: 32 * a + K, qc: qc + 32],
                            rhs=wtile[32 * a: 32 * a + K, off: off + 13],
                            start=False, stop=False,
                            tile_position=(32 * a, 32 * a))

            def emit_p2(j, q, zf, t0, t1, qbase):
                K, off, Nj = BLK_K[j], int(BLK_OFF[j]), BLK_N[j]
                last = (j == 7)
                if Nj <= 13:
                    return
                for t in range(t1 - NSPL, t1):
                    base = 128 * t + 13 * j
                    qc = 32 * (t - qbase)
                    for a in range(4):
                        nc.tensor.matmul(
                            zf[32 * a: 32 * a + 32, base + 13: base + Nj],
                            lhsT=q[32 * a: 32 * a + K, qc: qc + 32],
                            rhs=wtile[32 * a: 32 * a + K, off + 13: off + Nj],
                            start=False, stop=last,
                            tile_position=(32 * a, 32 * a))

            def load_x(g):
                xo = xpool.tile([128, 32 * t_slots], f32, tag="xo")
                xo3 = xo[:].rearrange("p (t w) -> p t w", w=32)
                nc.sync.dma_start(xo3[:, :, 0:8], x_r[:, g, :, :])
                out = []
                for h in (0, 1):
                    qt = qpool.tile([128, 32 * H], f32, tag=f"qx{h}")
                    nc.vector.transpose(qt[:], xo[:, 32 * H * h: 32 * H * (h + 1)])
                    out.append(qt)
                return out

            qx_next = [None]

            for gi in range(ng):
                zft = zpool.tile([128, 128 * t_slots], f32, tag="zf")
                zf = zft[:]
                zf4 = zf.rearrange("p (t q) -> p t q", q=128)
                # pre-zero PSUM via dummy bf16 matmuls: start=True clears
                # has_written at BANK granularity, so per-region start flags
                # on the real matmuls corrupt neighbouring slots in the bank.
                for b in range((128 * t_slots) // 512):
                    nc.tensor.matmul(zf[:, 512 * b: 512 * (b + 1)],
                                     lhsT=zl[:], rhs=zr[:],
                                     start=True, stop=True)

                # x for THIS pair was DMA'd/transposed during the previous
                # pair (software pipeline); prologue handles gi == 0.
                if gi == 0:
                    qx_cur = load_x(0)
                else:
                    qx_cur = qx_next[0]

                acc = fpool.tile([128, t_slots], f32, tag="acc")

                qprev = qx_cur
                for i in range(1, 9):
                    if i == 2 and gi + 1 < ng:
                        # prefetch next pair's x while DVE waits on this
                        # pair's early matmuls
                        qx_next[0] = load_x(gi + 1)
                    j = i - 1
                    emit_fp1(j, qprev[0][:], zf, 0, H, 0)
                    emit_fp1(j, qprev[1][:], zf, H, t_slots, H)
                    emit_p2(j, qprev[0][:], zf, 0, H, 0)
                    emit_p2(j, qprev[1][:], zf, H, t_slots, H)

                    qnext = []
                    for h in (0, 1):
                        hs = slice(H * h, H * (h + 1))
                        zf4h = zf4[:, hs, :]
                        if i < 8:
                            bct = bcpool.tile([128, 8 * H], f32, tag=f"bc{h}")
                            bc3 = bct[:].rearrange("p (t c) -> p t c", c=8)
                            nc.scalar.copy(bc3, zf4h[:, :, 13 * (i - 1): 13 * (i - 1) + 8])
                            slab3 = None
                        else:
                            slabt = slpool.tile([128, 13 * H], f32, tag=f"slab{h}")
                            slab3 = slabt[:].rearrange("p (t c) -> p t c", c=13)
                            nc.scalar.copy(slab3, zf4h[:, :, 91:104])
                            bc3 = None
                        ot = opool.tile([128, 32 * H], f32, tag=f"ot{h}")
                        ot3 = ot[:].rearrange("p (t w) -> p t w", w=32)
                        _emit_ops(nc, spool, zf4h, bc3, slab3, ot3,
                                  acc[:, hs], wf[:], consts, i, h)
                        if debug and gi == 0:
                            nc.sync.dma_start(
                                dbg[f"ot{i}"].ap().rearrange(
                                    "p (h c) -> p h c", h=2)[:, h, :], ot[:])
                        if i < 8:
                            qn = qpool.tile([128, 32 * H], f32, tag=f"qt{h}")
                            nc.vector.transpose(qn[:], ot[:])
                            qnext.append(qn)
                    qprev = qnext

                if debug and gi == 0:
                    zstage = opool.tile([128, 128 * t_slots], f32, tag="zdbg")
                    nc.scalar.copy(zstage[:], zf)
                    nc.sync.dma_start(dbg["zf"].ap(), zstage[:])
                nc.sync.dma_start(y_r[:, gi, :, 0], acc[:])

    nc.compile()
    return nc


def _get_program(key, bc, t_slots, ng):
    if key not in _PROG_CACHE:
        _PROG_CACHE[key] = _build_program(bc, t_slots, ng)
    return _PROG_CACHE[key]


def _in_maps(x, Ws, Wf):
    ws = _build_wstream(Ws, Wf)
    wfb = _build_wf(Wf)
    return [
        {"x": np.ascontiguousarray(x[c * BC:(c + 1) * BC]), "ws": ws,
         "wf": wfb}
        for c in range(N_CORES)
    ]


def kernel(**inputs):
    x = np.ascontiguousarray(np.asarray(inputs["x"], dtype=np.float32))
    Ws = [np.asarray(inputs[f"W{i}"], dtype=np.float32) for i in range(1, 9)]
    Wf = np.asarray(inputs["Wf"], dtype=np.float32)
    assert x.shape == (B_FULL, 8), x.shape

    nc = _get_program("full", BC, T, NG)
    res = run_bass_kernel_spmd(nc, _in_maps(x, Ws, Wf), list(range(N_CORES)))
    out = np.concatenate([res.results[c]["y"] for c in range(N_CORES)], axis=0)
    return out.astype(np.float32)


def profile_run(x, Ws, Wf, trace=True, tmpdir=None, trace_cores=None):
    """Timing/trace helper for test.py (not used by the grading harness)."""
    nc = _get_program("full", BC, T, NG)
    res = run_bass_kernel_spmd(nc, _in_maps(x, Ws, Wf), list(range(N_CORES)),
                               trace=trace, tmpdir=tmpdir,
                               trace_cores=trace_cores)
    return res
